# revision 32
# baseline (speedup 1.0000x reference)
"""Trainium2 Bass kernel for nn_MoEDiscriminator (8 experts, MLP 64->256->256->1).

Strategy (data-parallel over 8 NeuronCores):
- st [65536, 64] is sharded along batch: 8192 rows per core; expert weights
  are replicated on every core.
- All matmul operands are bf16 (host-converted); PSUM accumulates fp32.
  Measured end-to-end error vs the fp32 reference is ~4e-3 relative.
- Activations live as [feature_on_partitions, batch_on_free] SBUF tiles.
- Per expert c, per batch tile of 512:
    L1: K=64, so the two hidden halves run as a row-tiled PAIR: half 0 at
        PE rows 0-63, half 1 at rows 64-127 (st is duplicated across the
        two row groups). The two matmuls execute concurrently -> 2x L1.
    L2: 4 full 128x128 matmuls (2 out-halves x 2 k-chunks).
    L3: M=1 per expert; experts are col-tiled 4-at-a-time at PE columns
        0/32/64/96 (round-robin issue) -> 4 concurrent matmuls -> 4x L3.
        The [experts-on-partitions 0/32/64/96, 512] PSUM tile is DMA'd
        straight to DRAM with a partition-strided access pattern.
- relu(x + b) evictions run on ScalarE/VectorE, load-balanced; b3 is added
  on the host; output is reassembled host-side to [65536, 8, 1].
"""

import sys

sys.path.insert(0, "/opt/trn_rl_repo")
from contextlib import ExitStack

import numpy as np
import ml_dtypes

import concourse.bass as bass
import concourse.tile as tile
from concourse import bacc, mybir
from concourse.bass import ts
from concourse.bass_utils import run_bass_kernel_spmd

P = 128
C = 8            # experts
DS = 64          # input feature dim
H = 256          # hidden width
B = 65536        # full batch
NCORES = 8
NB = B // NCORES  # 8192 rows per core
BT = 512         # batch tile (free dim of matmuls)
NT = NB // BT    # 16
ST_CHUNKS = (512, 1536, 2048, 4096)   # graduated st chunk widths
PSUM_BUFS = (3, 2, 1)                 # (psumA, psumB pairs, psumD)
H1_BUFS = 12
H2_BUFS = 22
ACT_EXTRA = 0     # if > 0: every ACT_EXTRA-th item, ACT also takes L2 h1 evict

f32 = mybir.dt.float32
bf16 = mybir.dt.bfloat16
AF = mybir.ActivationFunctionType
ALU = mybir.AluOpType
BF_NP = ml_dtypes.bfloat16

_NC_CACHE = {}


def _build_nc(repeats=1):
    key = (repeats, ST_CHUNKS, PSUM_BUFS, H1_BUFS, H2_BUFS, ACT_EXTRA)
    if key in _NC_CACHE:
        return _NC_CACHE[key]
    nc = bacc.Bacc("TRN2", target_bir_lowering=False, debug=False,
                   num_devices=NCORES)
    st_d = nc.dram_tensor("st", [P, NB], bf16, kind="ExternalInput").ap()
    # boot: W1pack[:, 0, :] ++ bitcast(b1h ++ b2h)
    boot_d = nc.dram_tensor("boot", [P, P + 4 * C * 2],
                            bf16, kind="ExternalInput").ap()
    w1_d = nc.dram_tensor("w1", [C, P, P], bf16, kind="ExternalInput").ap()
    w2_d = nc.dram_tensor("w2", [C, 2, 2, P, P], bf16,
                          kind="ExternalInput").ap()
    # w3[c, k, m, p]: expert c's k-chunk in column 16 * (c // 4) of a 32-wide
    # zero-padded stationary (full-width so the start=True matmul initializes
    # every partition of the bank). Both expert-quads accumulate into one PSUM bank:
    # expert c lands on partition 32 * (c % 4) + 16 * (c // 4), so a single
    # stride-16 partition copy evicts all 8 experts.
    w3_d = nc.dram_tensor("w3", [C, 2, 32, P], bf16, kind="ExternalInput").ap()
    d_d = nc.dram_tensor("d", [C, NB], f32, kind="ExternalOutput").ap()

    with tile.TileContext(nc) as tc, ExitStack() as ctx:
        const = ctx.enter_context(tc.tile_pool(name="const", bufs=2))
        work1 = ctx.enter_context(tc.tile_pool(name="work1", bufs=H1_BUFS))
        work2 = ctx.enter_context(tc.tile_pool(name="work2", bufs=H2_BUFS))
        psumA = ctx.enter_context(
            tc.tile_pool(name="psumA", bufs=PSUM_BUFS[0], space="PSUM"))
        psumB = ctx.enter_context(
            tc.tile_pool(name="psumB", bufs=PSUM_BUFS[1], space="PSUM"))
        psumD = ctx.enter_context(
            tc.tile_pool(name="psumD", bufs=PSUM_BUFS[2], space="PSUM"))

        def body():
            # Boot DMA first: expert-0 W1 pair + both bias tables, so the
            # first L1 matmul waits on a single small transfer.
            boot_sb = const.tile([P, P + 4 * C * 2], bf16)
            nc.sync.dma_start(boot_sb[:], boot_d)
            bias_f32 = boot_sb[:, P:P + 4 * C * 2].bitcast(f32)  # [128, 32]
            b1_sb = bias_f32[:, 0:2 * C]
            b2_sb = bias_f32[:, 2 * C:4 * C]

            st_sb, st_off = [], []
            off = 0
            for i, cols in enumerate(ST_CHUNKS):
                st_sb.append(const.tile([P, cols], bf16, name=f"st_sb{i}"))
                st_off.append(off)
                off += cols
            assert off == NB
            nc.sync.dma_start(st_sb[0][:], st_d[:, 0:ST_CHUNKS[0]])

            w1_sb = const.tile([P, C, P], bf16)
            w2_sb = const.tile([P, C, 2, 2, P], bf16)
            w3_sb = const.tile([P, C, 2, 32], bf16)
            d_sb = const.tile([P, NB], f32)  # experts live on partitions 16*r
            # Weight DMAs interleaved so w1[c]/w2[c] land before expert c's
            # L1/L2 reach them in the pipeline.
            nc.sync.dma_start(w1_sb[:, 1:4], w1_d[1:4].rearrange("c p f -> p c f"))
            nc.sync.dma_start(w2_sb[:, 0:1],
                              w2_d[0:1].rearrange("c k j p f -> p c k j f"))
            nc.sync.dma_start(w1_sb[:, 4:8], w1_d[4:8].rearrange("c p f -> p c f"))
            nc.sync.dma_start(w3_sb[:], w3_d.rearrange("c k m p -> p c k m"))
            nc.sync.dma_start(w2_sb[:, 1:2],
                              w2_d[1:2].rearrange("c k j p f -> p c k j f"))
            nc.sync.dma_start(w2_sb[:, 2:4],
                              w2_d[2:4].rearrange("c k j p f -> p c k j f"))
            nc.sync.dma_start(st_sb[1][:],
                              st_d[:, st_off[1]:st_off[1] + ST_CHUNKS[1]])
            nc.sync.dma_start(w2_sb[:, 4:6],
                              w2_d[4:6].rearrange("c k j p f -> p c k j f"))
            nc.sync.dma_start(w2_sb[:, 6:8],
                              w2_d[6:8].rearrange("c k j p f -> p c k j f"))
            for i in range(2, len(ST_CHUNKS)):
                nc.sync.dma_start(st_sb[i][:],
                                  st_d[:, st_off[i]:st_off[i] + ST_CHUNKS[i]])

            def st_slice(t, r):
                col = t * BT
                for i, o in enumerate(st_off):
                    if o <= col < o + ST_CHUNKS[i]:
                        return st_sb[i][64 * r:64 * r + 64,
                                        col - o:col - o + BT]
                raise AssertionError

            def w1_ap(c, r):
                if c == 0:
                    return boot_sb[64 * r:64 * r + 64, 0:P]
                return w1_sb[64 * r:64 * r + 64, c, :]

            h1s, h2s = {}, {}

            def stage_l1(t, c):
                pA = [psumA.tile([P, BT], f32, tag="pA", name=f"pA{r}")
                      for r in range(2)]
                for r in range(2):
                    nc.tensor.matmul(pA[r][:], w1_ap(c, r), st_slice(t, r),
                                     start=True, stop=True)
                h1 = [work1.tile([P, BT], bf16, tag="h1", name=f"h1_{r}")
                      for r in range(2)]
                nc.scalar.activation(h1[0][:], pA[0][:], AF.Relu,
                                     bias=b1_sb[:, 2 * c:2 * c + 1])
                nc.vector.tensor_scalar(h1[1][:], pA[1][:],
                                        b1_sb[:, 2 * c + 1:2 * c + 2],
                                        0.0, ALU.add, ALU.max)
                h1s[(t, c)] = h1

            quadq = []

            def stage_l2_group(tp, c):
                # One expert's L2 for BOTH tiles of pair tp: the two batch
                # tiles share per-partition biases, so each output half
                # evicts as a single [128, 1024] op spanning 2 PSUM banks.
                h1pair = [h1s.pop((2 * tp + tt, c)) for tt in range(2)]
                pB = [psumB.tile([P, 2 * BT], f32, tag="pB", name=f"pB{j}")
                      for j in range(2)]
                for j in range(2):
                    for tt in range(2):
                        for k in range(2):
                            nc.tensor.matmul(
                                pB[j][:, tt * BT:(tt + 1) * BT],
                                w2_sb[:, c, k, j, :], h1pair[tt][k][:],
                                start=(k == 0), stop=(k == 1))
                h2p = [work2.tile([P, 2 * BT], bf16, tag="h2", name=f"h2_{j}")
                       for j in range(2)]
                nc.scalar.activation(h2p[0][:], pB[0][:], AF.Relu,
                                     bias=b2_sb[:, 2 * c:2 * c + 1])
                nc.vector.tensor_scalar(h2p[1][:], pB[1][:],
                                        b2_sb[:, 2 * c + 1:2 * c + 2],
                                        0.0, ALU.add, ALU.max)
                h2s[(tp, c)] = h2p
                if c == 3:
                    quadq.append((2 * tp, 0))
                elif c == 7:
                    quadq.extend([(2 * tp, 1), (2 * tp + 1, 0),
                                  (2 * tp + 1, 1)])

            pDs = {}

            def stage_l3_quad(t, q):
                tp, tt = divmod(t, 2)
                if q == 0:
                    pDs[t] = psumD.tile([P, BT], f32, tag="pD", name="pD")
                pD = pDs[t]
                for k in range(2):
                    for j in range(4):
                        e = 4 * q + j
                        h2p = h2s[(tp, e)]
                        nc.tensor.matmul(pD[32 * j:32 * j + 32, :],
                                         w3_sb[:, e, k, :],
                                         h2p[k][:, tt * BT:(tt + 1) * BT],
                                         start=(q == 0 and k == 0),
                                         stop=(q == 1 and k == 1),
                                         tile_position=(0, 32 * j),
                                         skip_group_check=True)
                if tt == 1:
                    for j in range(4):
                        h2s.pop((tp, 4 * q + j))
                if q == 1:
                    pD = pDs.pop(t)
                    nc.scalar.copy(d_sb[0:113, ts(t, BT)], pD[0:113, :])

            # Slot order interleaves the two tiles of each pair
            # (t0c0, t1c0, t0c1, t1c1, ...) so L2 groups land every other
            # slot and engine load stays smooth.
            NG = NT // 2 * C                     # L2 groups, in (tp, c) order
            for s in range(2 * NG):
                tp, r = divmod(s, 2 * C)
                c, tt = divmod(r, 2)
                stage_l1(2 * tp + tt, c)
                if s >= 3 and s % 2 == 1:
                    stage_l2_group(*divmod((s - 3) // 2, C))
                if quadq:
                    stage_l3_quad(*quadq.pop(0))
            stage_l2_group(NT // 2 - 1, C - 1)
            while quadq:
                stage_l3_quad(*quadq.pop(0))
            for r in range(C):
                nc.sync.dma_start(d_d[r:r + 1, :], d_sb[16 * r:16 * r + 1, :])

        for _rep in range(repeats):
            body()

    nc.compile()
    _NC_CACHE[key] = nc
    return nc


def _prep_weights(W1, b1, W2, b2, W3):
    # W1p[c]: rows 0-63 = W1[c, :, 0:128], rows 64-127 = W1[c, :, 128:256]
    W1p = np.empty((C, P, P), BF_NP)
    for c in range(C):
        W1p[c, 0:DS, :] = W1[c][:, 0:P].astype(BF_NP)
        W1p[c, DS:P, :] = W1[c][:, P:2 * P].astype(BF_NP)
    W2r = np.ascontiguousarray(
        W2.reshape(C, 2, P, 2, P).transpose(0, 1, 3, 2, 4)
    ).astype(BF_NP)  # W2r[c,k,j] = W2[c, k*128:(k+1)*128, j*128:(j+1)*128]
    # W3r[c, k, m, :]: expert c's k-chunk in column 16 * (c // 4) of a
    # 32-wide zero-padded stationary (see w3_d comment in _build_nc).
    W3r = np.zeros((C, 2, 32, P), BF_NP)
    for c in range(C):
        W3r[c, :, 16 * (c // 4), :] = W3.reshape(C, 2, P)[c].astype(BF_NP)
    b1h = np.ascontiguousarray(b1.reshape(C * 2, P).T)  # [128, C*2] f32
    b2h = np.ascontiguousarray(b2.reshape(C * 2, P).T)
    bias = np.concatenate([b1h, b2h], axis=1).astype(np.float32)  # [128, 32]
    boot = np.concatenate(
        [np.ascontiguousarray(W1p[0]),
         np.ascontiguousarray(bias).view(BF_NP)], axis=1)  # [128, 192] bf16
    return W1p, W2r, W3r, boot


def _make_in_maps(st, W1, b1, W2, b2, W3):
    W1p, W2r, W3r, boot = _prep_weights(W1, b1, W2, b2, W3)
    in_maps = []
    for core in range(NCORES):
        shard = st[core * NB:(core + 1) * NB]            # [8192, 64]
        stT = np.ascontiguousarray(
            np.concatenate([shard.T, shard.T], axis=0).astype(BF_NP))
        in_maps.append({"st": stT, "boot": boot, "w1": W1p, "w2": W2r,
                        "w3": W3r})
    return in_maps


class _SpmdExec:
    """Reusable jitted shard_map executor for a compiled Bass module
    (mirrors concourse.bass2jax.run_bass_via_pjrt; verified bit-identical)."""

    def __init__(self, nc, n_cores):
        import jax
        from jax.sharding import Mesh, PartitionSpec
        from jax.experimental.shard_map import shard_map
        from concourse.bass2jax import (_bass_exec_p, partition_id_tensor,
                                        install_neuronx_cc_hook)

        install_neuronx_cc_hook()
        self.n_cores = n_cores
        in_names, out_names, out_avals = [], [], []
        pname = nc.partition_id_tensor.name if nc.partition_id_tensor else None
        for alloc in nc.m.functions[0].allocations:
            if not isinstance(alloc, mybir.MemoryLocationSet):
                continue
            name = alloc.memorylocations[0].name
            if alloc.kind == "ExternalInput":
                if name != pname:
                    in_names.append(name)
            elif alloc.kind == "ExternalOutput":
                out_names.append(name)
                out_avals.append(jax.core.ShapedArray(
                    tuple(alloc.tensor_shape), mybir.dt.np(alloc.dtype)))
        self.in_names, self.out_names, self.out_avals = \
            in_names, out_names, out_avals
        all_in = in_names + out_names + ([pname] if pname else [])

        def _bdy(*args):
            ops = list(args)
            if pname is not None:
                ops.append(partition_id_tensor())
            return tuple(_bass_exec_p.bind(
                *ops, out_avals=tuple(out_avals), in_names=tuple(all_in),
                out_names=tuple(out_names), lowering_input_output_aliases=(),
                sim_require_finite=True, sim_require_nnan=True, nc=nc))

        mesh = Mesh(np.asarray(jax.devices()[:n_cores]), ("core",))
        nio = len(in_names) + len(out_names)
        self.sharded = jax.jit(
            shard_map(_bdy, mesh=mesh,
                      in_specs=(PartitionSpec("core"),) * nio,
                      out_specs=(PartitionSpec("core"),) * len(out_names),
                      check_rep=False),
            keep_unused=True)

    def run(self, in_maps):
        args = [np.concatenate([np.asarray(m[n]) for m in in_maps], axis=0)
                for n in self.in_names]
        args += [np.zeros((self.n_cores * a.shape[0], *a.shape[1:]), a.dtype)
                 for a in self.out_avals]
        outs = self.sharded(*args)
        return [{n: np.asarray(outs[i]).reshape(
                    self.n_cores, *self.out_avals[i].shape)[c]
                 for i, n in enumerate(self.out_names)}
                for c in range(self.n_cores)]


_EXEC_CACHE = {}


def _run_spmd(nc, in_maps, first_call):
    """First call goes through bass_utils.run_bass_kernel_spmd; later calls
    reuse a cached PJRT executable (bit-identical output, no re-jit)."""
    if not first_call:
        ex = _EXEC_CACHE.get(id(nc))
        if ex is None:
            ex = _EXEC_CACHE[id(nc)] = _SpmdExec(nc, NCORES)
        return ex.run(in_maps)
    import os
    try:
        return run_bass_kernel_spmd(
            nc, in_maps, core_ids=list(range(NCORES))).results
    except ModuleNotFoundError:
        # BASS_TRACE set but the axon NTFF hook module is absent: force
        # trace off and retry.
        os.environ["BASS_NEVER_TRACE"] = "1"
        return run_bass_kernel_spmd(
            nc, in_maps, core_ids=list(range(NCORES))).results


_CALLED = False


def kernel(st, W1, b1, W2, b2, W3, b3):
    global _CALLED
    st = np.ascontiguousarray(np.asarray(st, np.float32))
    in_maps = _make_in_maps(
        st,
        np.asarray(W1, np.float32), np.asarray(b1, np.float32),
        np.asarray(W2, np.float32), np.asarray(b2, np.float32),
        np.asarray(W3, np.float32))
    nc = _build_nc(1)
    results = _run_spmd(nc, in_maps, first_call=not _CALLED)
    _CALLED = True

    b3v = np.asarray(b3, np.float32).reshape(1, C)
    # d row r holds expert 4 * (r % 2) + r // 2 (PSUM partition 16 * r maps
    # to col group r // 2, quad r % 2).
    perm = [4 * (r % 2) + r // 2 for r in range(C)]
    out = np.empty((B, C, 1), np.float32)
    for core in range(NCORES):
        d = results[core]["d"]                            # [8, 8192]
        out[core * NB:(core + 1) * NB, :, 0][:, perm] = d.T
    out[:, :, 0] += b3v
    return out


# revision 35
# speedup vs baseline: 1.0254x; 1.0254x over previous
"""Trainium2 Bass kernel for nn_MoEDiscriminator (8 experts, MLP 64->256->256->1).

Strategy (data-parallel over 8 NeuronCores):
- st [65536, 64] is sharded along batch: 8192 rows per core; expert weights
  are replicated on every core.
- All matmul operands are bf16 (host-converted); PSUM accumulates fp32.
  Measured end-to-end error vs the fp32 reference is ~4e-3 relative.
- Activations live as [feature_on_partitions, batch_on_free] SBUF tiles.
- Per expert c, per batch tile of 512:
    L1: K=64, so the two hidden halves run as a row-tiled PAIR: half 0 at
        PE rows 0-63, half 1 at rows 64-127 (st is duplicated across the
        two row groups). The two matmuls execute concurrently -> 2x L1.
    L2: 4 full 128x128 matmuls (2 out-halves x 2 k-chunks).
    L3: M=1 per expert; experts are col-tiled 4-at-a-time at PE columns
        0/32/64/96 (round-robin issue) -> 4 concurrent matmuls -> 4x L3.
        The [experts-on-partitions 0/32/64/96, 512] PSUM tile is DMA'd
        straight to DRAM with a partition-strided access pattern.
- relu(x + b) evictions run on ScalarE/VectorE, load-balanced; b3 is added
  on the host; output is reassembled host-side to [65536, 8, 1].
"""

import sys

sys.path.insert(0, "/opt/trn_rl_repo")
from contextlib import ExitStack

import numpy as np
import ml_dtypes

import concourse.bass as bass
import concourse.tile as tile
from concourse import bacc, mybir
from concourse.bass import ts
from concourse.bass_utils import run_bass_kernel_spmd

P = 128
C = 8            # experts
DS = 64          # input feature dim
H = 256          # hidden width
B = 65536        # full batch
NCORES = 8
NB = B // NCORES  # 8192 rows per core
BT = 512         # batch tile (free dim of matmuls)
NT = NB // BT    # 16
ST_CHUNKS = (512, 1536, 2048, 4096)   # graduated st chunk widths
PSUM_BUFS = (2, 3, 1)                 # (psumA pairs, psumB, psumD)
H1_BUFS = 4
H2_BUFS = 10
ACT_EXTRA = 0     # if > 0: every ACT_EXTRA-th item, ACT also takes L2 h1 evict

f32 = mybir.dt.float32
bf16 = mybir.dt.bfloat16
AF = mybir.ActivationFunctionType
ALU = mybir.AluOpType
BF_NP = ml_dtypes.bfloat16

_NC_CACHE = {}


def _build_nc(repeats=1):
    key = (repeats, ST_CHUNKS, PSUM_BUFS, H1_BUFS, H2_BUFS, ACT_EXTRA)
    if key in _NC_CACHE:
        return _NC_CACHE[key]
    nc = bacc.Bacc("TRN2", target_bir_lowering=False, debug=False,
                   num_devices=NCORES)
    # st rows: 0-63 features, row 64 ones (bias row), 65-95 zero. K=96 L1
    # matmuls fold b1 into the stationary, so L1 evictions are bias-free and
    # the two hidden halves evict as one [128, 1024] relu-only op.
    st_d = nc.dram_tensor("st", [96, NB], bf16, kind="ExternalInput").ap()
    # boot: W1pack[0, h0] ++ W1pack[0, h1] ++ bitcast(b2h)
    boot_d = nc.dram_tensor("boot", [P, 2 * P + 2 * C * 2],
                            bf16, kind="ExternalInput").ap()
    w1_d = nc.dram_tensor("w1", [C, 2, 96, P], bf16, kind="ExternalInput").ap()
    w2_d = nc.dram_tensor("w2", [C, 2, 2, P, P], bf16,
                          kind="ExternalInput").ap()
    # w3[c, k, m, p]: expert c's k-chunk in column 16 * (c // 4) of a 32-wide
    # zero-padded stationary (full-width so the start=True matmul initializes
    # every partition of the bank). Both expert-quads accumulate into one PSUM bank:
    # expert c lands on partition 32 * (c % 4) + 16 * (c // 4), so a single
    # stride-16 partition copy evicts all 8 experts.
    w3_d = nc.dram_tensor("w3", [C, 2, 32, P], bf16, kind="ExternalInput").ap()
    d_d = nc.dram_tensor("d", [C, NB], f32, kind="ExternalOutput").ap()

    with tile.TileContext(nc) as tc, ExitStack() as ctx:
        const = ctx.enter_context(tc.tile_pool(name="const", bufs=2))
        work1 = ctx.enter_context(tc.tile_pool(name="work1", bufs=H1_BUFS))
        work2 = ctx.enter_context(tc.tile_pool(name="work2", bufs=H2_BUFS))
        psumA = ctx.enter_context(
            tc.tile_pool(name="psumA", bufs=PSUM_BUFS[0], space="PSUM"))
        psumB = ctx.enter_context(
            tc.tile_pool(name="psumB", bufs=PSUM_BUFS[1], space="PSUM"))
        psumD = ctx.enter_context(
            tc.tile_pool(name="psumD", bufs=PSUM_BUFS[2], space="PSUM"))

        def body():
            # Boot DMA first: expert-0 W1 pair + both bias tables, so the
            # first L1 matmul waits on a single small transfer.
            boot_sb = const.tile([P, 2 * P + 2 * C * 2], bf16)
            nc.sync.dma_start(boot_sb[:], boot_d)
            b2_sb = boot_sb[:, 2 * P:2 * P + 2 * C * 2].bitcast(f32)

            st_sb, st_off = [], []
            off = 0
            for i, cols in enumerate(ST_CHUNKS):
                st_sb.append(const.tile([96, cols], bf16, name=f"st_sb{i}"))
                st_off.append(off)
                off += cols
            assert off == NB
            nc.sync.dma_start(st_sb[0][:], st_d[:, 0:ST_CHUNKS[0]])

            w1_sb = const.tile([96, C, 2, P], bf16)
            w2_sb = const.tile([P, C, 2, 2, P], bf16)
            w3_sb = const.tile([P, C, 2, 32], bf16)
            d_sb = const.tile([P, NB], f32)  # experts live on partitions 16*r
            # Weight DMAs interleaved so w1[c]/w2[c] land before expert c's
            # L1/L2 reach them in the pipeline.
            nc.sync.dma_start(w1_sb[:, 1:4],
                              w1_d[1:4].rearrange("c h p f -> p c h f"))
            nc.sync.dma_start(w2_sb[:, 0:1],
                              w2_d[0:1].rearrange("c k j p f -> p c k j f"))
            nc.sync.dma_start(w1_sb[:, 4:8],
                              w1_d[4:8].rearrange("c h p f -> p c h f"))
            nc.sync.dma_start(w3_sb[:], w3_d.rearrange("c k m p -> p c k m"))
            nc.sync.dma_start(w2_sb[:, 1:2],
                              w2_d[1:2].rearrange("c k j p f -> p c k j f"))
            nc.sync.dma_start(w2_sb[:, 2:4],
                              w2_d[2:4].rearrange("c k j p f -> p c k j f"))
            nc.sync.dma_start(st_sb[1][:],
                              st_d[:, st_off[1]:st_off[1] + ST_CHUNKS[1]])
            nc.sync.dma_start(w2_sb[:, 4:6],
                              w2_d[4:6].rearrange("c k j p f -> p c k j f"))
            nc.sync.dma_start(w2_sb[:, 6:8],
                              w2_d[6:8].rearrange("c k j p f -> p c k j f"))
            for i in range(2, len(ST_CHUNKS)):
                nc.sync.dma_start(st_sb[i][:],
                                  st_d[:, st_off[i]:st_off[i] + ST_CHUNKS[i]])

            def st_slice(t):
                col = t * BT
                for i, o in enumerate(st_off):
                    if o <= col < o + ST_CHUNKS[i]:
                        return st_sb[i][:, col - o:col - o + BT]
                raise AssertionError

            def w1_ap(c, h):
                if c == 0:
                    return boot_sb[0:96, h * P:(h + 1) * P]
                return w1_sb[:, c, h, :]

            h1s, h2s = {}, {}

            def stage_l1(i):
                t, c = divmod(i, C)
                pA = psumA.tile([P, 2 * BT], f32, tag="pA", name="pA")
                for h in range(2):
                    nc.tensor.matmul(pA[:, h * BT:(h + 1) * BT],
                                     w1_ap(c, h), st_slice(t),
                                     start=True, stop=True)
                h1 = work1.tile([P, 2 * BT], bf16, tag="h1", name="h1")
                if i % 2 == 0:
                    nc.scalar.activation(h1[:], pA[:], AF.Relu)
                else:
                    nc.vector.tensor_scalar_max(h1[:], pA[:], 0.0)
                h1s[i] = h1

            def stage_l2(i):
                t, c = divmod(i, C)
                h1 = h1s.pop(i)
                pB = [psumB.tile([P, BT], f32, tag="pB", name=f"pB{j}")
                      for j in range(2)]
                for j in range(2):
                    for k in range(2):
                        nc.tensor.matmul(pB[j][:], w2_sb[:, c, k, j, :],
                                         h1[:, k * BT:(k + 1) * BT],
                                         start=(k == 0), stop=(k == 1))
                h2 = [work2.tile([P, BT], bf16, tag="h2", name=f"h2_{j}")
                      for j in range(2)]
                nc.scalar.activation(h2[0][:], pB[0][:], AF.Relu,
                                     bias=b2_sb[:, 2 * c:2 * c + 1])
                if ACT_EXTRA and i % ACT_EXTRA == 0:
                    nc.scalar.activation(h2[1][:], pB[1][:], AF.Relu,
                                         bias=b2_sb[:, 2 * c + 1:2 * c + 2])
                else:
                    nc.vector.tensor_scalar(h2[1][:], pB[1][:],
                                            b2_sb[:, 2 * c + 1:2 * c + 2],
                                            0.0, ALU.add, ALU.max)
                h2s[i] = h2

            pDs = {}

            def stage_l3_quad(m):
                # m = item index of the quad's last expert (c % 4 == 3)
                t, c_last = divmod(m, C)
                q = c_last // 4
                if q == 0:
                    pDs[t] = psumD.tile([P, BT], f32, tag="pD", name="pD")
                pD = pDs[t]
                for k in range(2):
                    for j in range(4):
                        e = 4 * q + j
                        h2 = h2s[t * C + e]
                        nc.tensor.matmul(pD[32 * j:32 * j + 32, :],
                                         w3_sb[:, e, k, :], h2[k][:],
                                         start=(q == 0 and k == 0),
                                         stop=(q == 1 and k == 1),
                                         tile_position=(0, 32 * j),
                                         skip_group_check=True)
                for j in range(4):
                    h2s.pop(t * C + 4 * q + j)
                if q == 1:
                    pD = pDs.pop(t)
                    if t % 2 == 0:
                        nc.scalar.copy(d_sb[0:113, ts(t, BT)], pD[0:113, :])
                    else:
                        nc.vector.tensor_copy(d_sb[0:113, ts(t, BT)],
                                              pD[0:113, :])

            N = NT * C
            for i in range(N):
                stage_l1(i)
                if i >= 1:
                    stage_l2(i - 1)
                m = i - 2
                if m >= 0 and m % 4 == 3:
                    stage_l3_quad(m)
            stage_l2(N - 1)
            stage_l3_quad(N - 1)
            for r in range(C):
                nc.sync.dma_start(d_d[r:r + 1, :], d_sb[16 * r:16 * r + 1, :])

        for _rep in range(repeats):
            body()

    nc.compile()
    _NC_CACHE[key] = nc
    return nc


def _prep_weights(W1, b1, W2, b2, W3):
    # W1p[c, h]: rows 0-63 = W1[c, :, 128h:128h+128], row 64 = b1 slice
    # (bias rides the K=96 matmul against st's ones-row), rows 65-95 zero.
    W1p = np.zeros((C, 2, 96, P), BF_NP)
    for c in range(C):
        for h in range(2):
            W1p[c, h, 0:DS, :] = W1[c][:, h * P:(h + 1) * P].astype(BF_NP)
            W1p[c, h, DS, :] = b1[c, h * P:(h + 1) * P].astype(BF_NP)
    W2r = np.ascontiguousarray(
        W2.reshape(C, 2, P, 2, P).transpose(0, 1, 3, 2, 4)
    ).astype(BF_NP)  # W2r[c,k,j] = W2[c, k*128:(k+1)*128, j*128:(j+1)*128]
    # W3r[c, k, m, :]: expert c's k-chunk in column 16 * (c // 4) of a
    # 32-wide zero-padded stationary (see w3_d comment in _build_nc).
    W3r = np.zeros((C, 2, 32, P), BF_NP)
    for c in range(C):
        W3r[c, :, 16 * (c // 4), :] = W3.reshape(C, 2, P)[c].astype(BF_NP)
    b2h = np.ascontiguousarray(b2.reshape(C * 2, P).T).astype(np.float32)
    w1c0 = np.zeros((P, 2 * P), BF_NP)
    for h in range(2):
        w1c0[0:96, h * P:(h + 1) * P] = W1p[0, h]
    boot = np.concatenate(
        [w1c0, np.ascontiguousarray(b2h).view(BF_NP)], axis=1)  # [128, 288]
    return W1p, W2r, W3r, boot


def _make_in_maps(st, W1, b1, W2, b2, W3):
    W1p, W2r, W3r, boot = _prep_weights(W1, b1, W2, b2, W3)
    in_maps = []
    ones = np.ones((1, NB), np.float32)
    zeros = np.zeros((31, NB), np.float32)
    for core in range(NCORES):
        shard = st[core * NB:(core + 1) * NB]            # [8192, 64]
        stT = np.ascontiguousarray(np.concatenate(
            [shard.T, ones, zeros], axis=0).astype(BF_NP))  # [96, 8192]
        in_maps.append({"st": stT, "boot": boot, "w1": W1p, "w2": W2r,
                        "w3": W3r})
    return in_maps


class _SpmdExec:
    """Reusable jitted shard_map executor for a compiled Bass module
    (mirrors concourse.bass2jax.run_bass_via_pjrt; verified bit-identical)."""

    def __init__(self, nc, n_cores):
        import jax
        from jax.sharding import Mesh, PartitionSpec
        from jax.experimental.shard_map import shard_map
        from concourse.bass2jax import (_bass_exec_p, partition_id_tensor,
                                        install_neuronx_cc_hook)

        install_neuronx_cc_hook()
        self.n_cores = n_cores
        in_names, out_names, out_avals = [], [], []
        pname = nc.partition_id_tensor.name if nc.partition_id_tensor else None
        for alloc in nc.m.functions[0].allocations:
            if not isinstance(alloc, mybir.MemoryLocationSet):
                continue
            name = alloc.memorylocations[0].name
            if alloc.kind == "ExternalInput":
                if name != pname:
                    in_names.append(name)
            elif alloc.kind == "ExternalOutput":
                out_names.append(name)
                out_avals.append(jax.core.ShapedArray(
                    tuple(alloc.tensor_shape), mybir.dt.np(alloc.dtype)))
        self.in_names, self.out_names, self.out_avals = \
            in_names, out_names, out_avals
        all_in = in_names + out_names + ([pname] if pname else [])

        def _bdy(*args):
            ops = list(args)
            if pname is not None:
                ops.append(partition_id_tensor())
            return tuple(_bass_exec_p.bind(
                *ops, out_avals=tuple(out_avals), in_names=tuple(all_in),
                out_names=tuple(out_names), lowering_input_output_aliases=(),
                sim_require_finite=True, sim_require_nnan=True, nc=nc))

        mesh = Mesh(np.asarray(jax.devices()[:n_cores]), ("core",))
        nio = len(in_names) + len(out_names)
        self.sharded = jax.jit(
            shard_map(_bdy, mesh=mesh,
                      in_specs=(PartitionSpec("core"),) * nio,
                      out_specs=(PartitionSpec("core"),) * len(out_names),
                      check_rep=False),
            keep_unused=True)

    def run(self, in_maps):
        args = [np.concatenate([np.asarray(m[n]) for m in in_maps], axis=0)
                for n in self.in_names]
        args += [np.zeros((self.n_cores * a.shape[0], *a.shape[1:]), a.dtype)
                 for a in self.out_avals]
        outs = self.sharded(*args)
        return [{n: np.asarray(outs[i]).reshape(
                    self.n_cores, *self.out_avals[i].shape)[c]
                 for i, n in enumerate(self.out_names)}
                for c in range(self.n_cores)]


_EXEC_CACHE = {}


def _run_spmd(nc, in_maps, first_call):
    """First call goes through bass_utils.run_bass_kernel_spmd; later calls
    reuse a cached PJRT executable (bit-identical output, no re-jit)."""
    if not first_call:
        ex = _EXEC_CACHE.get(id(nc))
        if ex is None:
            ex = _EXEC_CACHE[id(nc)] = _SpmdExec(nc, NCORES)
        return ex.run(in_maps)
    import os
    try:
        return run_bass_kernel_spmd(
            nc, in_maps, core_ids=list(range(NCORES))).results
    except ModuleNotFoundError:
        # BASS_TRACE set but the axon NTFF hook module is absent: force
        # trace off and retry.
        os.environ["BASS_NEVER_TRACE"] = "1"
        return run_bass_kernel_spmd(
            nc, in_maps, core_ids=list(range(NCORES))).results


_CALLED = False


def kernel(st, W1, b1, W2, b2, W3, b3):
    global _CALLED
    st = np.ascontiguousarray(np.asarray(st, np.float32))
    in_maps = _make_in_maps(
        st,
        np.asarray(W1, np.float32), np.asarray(b1, np.float32),
        np.asarray(W2, np.float32), np.asarray(b2, np.float32),
        np.asarray(W3, np.float32))
    nc = _build_nc(1)
    results = _run_spmd(nc, in_maps, first_call=not _CALLED)
    _CALLED = True

    b3v = np.asarray(b3, np.float32).reshape(1, C)
    # d row r holds expert 4 * (r % 2) + r // 2 (PSUM partition 16 * r maps
    # to col group r // 2, quad r % 2).
    perm = [4 * (r % 2) + r // 2 for r in range(C)]
    out = np.empty((B, C, 1), np.float32)
    for core in range(NCORES):
        d = results[core]["d"]                            # [8, 8192]
        out[core * NB:(core + 1) * NB, :, 0][:, perm] = d.T
    out[:, :, 0] += b3v
    return out


# revision 37
# speedup vs baseline: 1.2176x; 1.1875x over previous
"""Trainium2 Bass kernel for nn_MoEDiscriminator (8 experts, MLP 64->256->256->1).

Strategy (data-parallel over 8 NeuronCores):
- st [65536, 64] is sharded along batch: 8192 rows per core; expert weights
  are replicated on every core.
- All matmul operands are bf16 (host-converted); PSUM accumulates fp32.
  Measured end-to-end error vs the fp32 reference is ~4e-3 relative.
- Activations live as [feature_on_partitions, batch_on_free] SBUF tiles.
- Per expert c, per batch tile of 512:
    L1: K=64, so the two hidden halves run as a row-tiled PAIR: half 0 at
        PE rows 0-63, half 1 at rows 64-127 (st is duplicated across the
        two row groups). The two matmuls execute concurrently -> 2x L1.
    L2: 4 full 128x128 matmuls (2 out-halves x 2 k-chunks).
    L3: M=1 per expert; experts are col-tiled 4-at-a-time at PE columns
        0/32/64/96 (round-robin issue) -> 4 concurrent matmuls -> 4x L3.
        The [experts-on-partitions 0/32/64/96, 512] PSUM tile is DMA'd
        straight to DRAM with a partition-strided access pattern.
- relu(x + b) evictions run on ScalarE/VectorE, load-balanced; b3 is added
  on the host; output is reassembled host-side to [65536, 8, 1].
"""

import sys

sys.path.insert(0, "/opt/trn_rl_repo")
from contextlib import ExitStack

import numpy as np
import ml_dtypes

import concourse.bass as bass
import concourse.tile as tile
from concourse import bacc, mybir
from concourse.bass import ts
from concourse.bass_utils import run_bass_kernel_spmd

P = 128
C = 8            # experts
DS = 64          # input feature dim
H = 256          # hidden width
B = 65536        # full batch
NCORES = 8
NB = B // NCORES  # 8192 rows per core
BT = 512         # batch tile (free dim of matmuls)
NT = NB // BT    # 16
ST_CHUNKS = (512, 1536, 2048, 4096)   # graduated st chunk widths
PSUM_BUFS = (3, 3, 2)                 # (psumA, psumB, psumD)
H1_BUFS = 4
H2_BUFS = 10
ACT_EXTRA = 0     # if > 0: every ACT_EXTRA-th item, ACT also takes L2 h1 evict

f32 = mybir.dt.float32
bf16 = mybir.dt.bfloat16
AF = mybir.ActivationFunctionType
ALU = mybir.AluOpType
BF_NP = ml_dtypes.bfloat16

_NC_CACHE = {}


def _build_nc(repeats=1):
    key = (repeats, ST_CHUNKS, PSUM_BUFS, H1_BUFS, H2_BUFS, ACT_EXTRA)
    if key in _NC_CACHE:
        return _NC_CACHE[key]
    nc = bacc.Bacc("TRN2", target_bir_lowering=False, debug=False,
                   num_devices=NCORES)
    st_d = nc.dram_tensor("st", [P, NB], bf16, kind="ExternalInput").ap()
    # boot: W1pack[:, 0, :] ++ bitcast(b1h ++ b2h)
    boot_d = nc.dram_tensor("boot", [P, P + 4 * C * 2],
                            bf16, kind="ExternalInput").ap()
    w1_d = nc.dram_tensor("w1", [C, P, P], bf16, kind="ExternalInput").ap()
    w2_d = nc.dram_tensor("w2", [C, 2, 2, P, P], bf16,
                          kind="ExternalInput").ap()
    # w3[c, k, m, p]: expert c's k-chunk in column 16 * (c // 4) of a 32-wide
    # zero-padded stationary (full-width so the start=True matmul initializes
    # every partition of the bank). Both expert-quads accumulate into one PSUM bank:
    # expert c lands on partition 32 * (c % 4) + 16 * (c // 4), so a single
    # stride-16 partition copy evicts all 8 experts.
    w3_d = nc.dram_tensor("w3", [C, 2, 32, P], bf16, kind="ExternalInput").ap()
    d_d = nc.dram_tensor("d", [C, NB], f32, kind="ExternalOutput").ap()

    with tile.TileContext(nc) as tc, ExitStack() as ctx:
        const = ctx.enter_context(tc.tile_pool(name="const", bufs=2))
        work1 = ctx.enter_context(tc.tile_pool(name="work1", bufs=H1_BUFS))
        work2 = ctx.enter_context(tc.tile_pool(name="work2", bufs=H2_BUFS))
        psumA = ctx.enter_context(
            tc.tile_pool(name="psumA", bufs=PSUM_BUFS[0], space="PSUM"))
        psumB = ctx.enter_context(
            tc.tile_pool(name="psumB", bufs=PSUM_BUFS[1], space="PSUM"))
        psumD = ctx.enter_context(
            tc.tile_pool(name="psumD", bufs=PSUM_BUFS[2], space="PSUM"))

        def body():
            # Boot DMA first: expert-0 W1 pair + both bias tables, so the
            # first L1 matmul waits on a single small transfer.
            boot_sb = const.tile([P, P + 4 * C * 2], bf16)
            nc.sync.dma_start(boot_sb[:], boot_d)
            bias_f32 = boot_sb[:, P:P + 4 * C * 2].bitcast(f32)  # [128, 32]
            b1_sb = bias_f32[:, 0:2 * C]
            b2_sb = bias_f32[:, 2 * C:4 * C]

            st_sb, st_off = [], []
            off = 0
            for i, cols in enumerate(ST_CHUNKS):
                st_sb.append(const.tile([P, cols], bf16, name=f"st_sb{i}"))
                st_off.append(off)
                off += cols
            assert off == NB
            nc.sync.dma_start(st_sb[0][:], st_d[:, 0:ST_CHUNKS[0]])

            w1_sb = const.tile([P, C, P], bf16)
            w2_sb = const.tile([P, C, 2, 2, P], bf16)
            w3_sb = const.tile([P, C, 2, 32], bf16)
            d_sb = const.tile([P, NB], f32)  # experts live on partitions 16*r
            # Weight DMAs interleaved so w1[c]/w2[c] land before expert c's
            # L1/L2 reach them in the pipeline.
            nc.sync.dma_start(w1_sb[:, 1:4], w1_d[1:4].rearrange("c p f -> p c f"))
            nc.sync.dma_start(w2_sb[:, 0:1],
                              w2_d[0:1].rearrange("c k j p f -> p c k j f"))
            nc.sync.dma_start(w1_sb[:, 4:8], w1_d[4:8].rearrange("c p f -> p c f"))
            nc.sync.dma_start(w3_sb[:], w3_d.rearrange("c k m p -> p c k m"))
            nc.sync.dma_start(w2_sb[:, 1:2],
                              w2_d[1:2].rearrange("c k j p f -> p c k j f"))
            nc.sync.dma_start(w2_sb[:, 2:4],
                              w2_d[2:4].rearrange("c k j p f -> p c k j f"))
            nc.sync.dma_start(st_sb[1][:],
                              st_d[:, st_off[1]:st_off[1] + ST_CHUNKS[1]])
            nc.sync.dma_start(w2_sb[:, 4:6],
                              w2_d[4:6].rearrange("c k j p f -> p c k j f"))
            nc.sync.dma_start(w2_sb[:, 6:8],
                              w2_d[6:8].rearrange("c k j p f -> p c k j f"))
            for i in range(2, len(ST_CHUNKS)):
                nc.sync.dma_start(st_sb[i][:],
                                  st_d[:, st_off[i]:st_off[i] + ST_CHUNKS[i]])

            def st_slice(t, r):
                col = t * BT
                for i, o in enumerate(st_off):
                    if o <= col < o + ST_CHUNKS[i]:
                        return st_sb[i][64 * r:64 * r + 64,
                                        col - o:col - o + BT]
                raise AssertionError

            def w1_ap(c, r):
                if c == 0:
                    return boot_sb[64 * r:64 * r + 64, 0:P]
                return w1_sb[64 * r:64 * r + 64, c, :]

            h1s, h2s = {}, {}

            def stage_l1(i):
                t, c = divmod(i, C)
                pA = [psumA.tile([P, BT], f32, tag="pA", name=f"pA{r}")
                      for r in range(2)]
                for r in range(2):
                    nc.tensor.matmul(pA[r][:], w1_ap(c, r), st_slice(t, r),
                                     start=True, stop=True)
                h1 = [work1.tile([P, BT], bf16, tag="h1", name=f"h1_{r}")
                      for r in range(2)]
                nc.scalar.activation(h1[0][:], pA[0][:], AF.Relu,
                                     bias=b1_sb[:, 2 * c:2 * c + 1])
                nc.vector.tensor_scalar(h1[1][:], pA[1][:],
                                        b1_sb[:, 2 * c + 1:2 * c + 2],
                                        0.0, ALU.add, ALU.max)
                h1s[i] = h1

            def stage_l2(i):
                t, c = divmod(i, C)
                h1 = h1s.pop(i)
                pB = [psumB.tile([P, BT], f32, tag="pB", name=f"pB{j}")
                      for j in range(2)]
                for j in range(2):
                    for k in range(2):
                        nc.tensor.matmul(pB[j][:], w2_sb[:, c, k, j, :],
                                         h1[k][:],
                                         start=(k == 0), stop=(k == 1))
                h2 = [work2.tile([P, BT], bf16, tag="h2", name=f"h2_{j}")
                      for j in range(2)]
                nc.scalar.activation(h2[0][:], pB[0][:], AF.Relu,
                                     bias=b2_sb[:, 2 * c:2 * c + 1])
                if ACT_EXTRA and i % ACT_EXTRA == 0:
                    nc.scalar.activation(h2[1][:], pB[1][:], AF.Relu,
                                         bias=b2_sb[:, 2 * c + 1:2 * c + 2])
                else:
                    nc.vector.tensor_scalar(h2[1][:], pB[1][:],
                                            b2_sb[:, 2 * c + 1:2 * c + 2],
                                            0.0, ALU.add, ALU.max)
                h2s[i] = h2

            pDs = {}

            def stage_l3_quad(m):
                # m = item index of the quad's last expert (c % 4 == 3)
                t, c_last = divmod(m, C)
                q = c_last // 4
                if q == 0:
                    pDs[t] = psumD.tile([P, BT], f32, tag="pD", name="pD")
                pD = pDs[t]
                for j in range(4):
                    for k in range(2):
                        e = 4 * q + j
                        h2 = h2s[t * C + e]
                        nc.tensor.matmul(pD[32 * j:32 * j + 32, :],
                                         w3_sb[:, e, k, :], h2[k][:],
                                         start=(q == 0 and k == 0),
                                         stop=(q == 1 and k == 1),
                                         tile_position=(0, 32 * j),
                                         skip_group_check=True)
                for j in range(4):
                    h2s.pop(t * C + 4 * q + j)
                if q == 1:
                    pD = pDs.pop(t)
                    nc.scalar.copy(d_sb[0:113, ts(t, BT)], pD[0:113, :])

            N = NT * C
            for i in range(N):
                stage_l1(i)
                if i >= 1:
                    stage_l2(i - 1)
                m = i - 2
                if m >= 0 and m % 4 == 3:
                    stage_l3_quad(m)
            stage_l2(N - 1)
            stage_l3_quad(N - 1)
            for r in range(C):
                nc.sync.dma_start(d_d[r:r + 1, :], d_sb[16 * r:16 * r + 1, :])

        for _rep in range(repeats):
            body()

    nc.compile()
    _NC_CACHE[key] = nc
    return nc


def _prep_weights(W1, b1, W2, b2, W3):
    # W1p[c]: rows 0-63 = W1[c, :, 0:128], rows 64-127 = W1[c, :, 128:256]
    W1p = np.empty((C, P, P), BF_NP)
    for c in range(C):
        W1p[c, 0:DS, :] = W1[c][:, 0:P].astype(BF_NP)
        W1p[c, DS:P, :] = W1[c][:, P:2 * P].astype(BF_NP)
    W2r = np.ascontiguousarray(
        W2.reshape(C, 2, P, 2, P).transpose(0, 1, 3, 2, 4)
    ).astype(BF_NP)  # W2r[c,k,j] = W2[c, k*128:(k+1)*128, j*128:(j+1)*128]
    # W3r[c, k, m, :]: expert c's k-chunk in column 16 * (c // 4) of a
    # 32-wide zero-padded stationary (see w3_d comment in _build_nc).
    W3r = np.zeros((C, 2, 32, P), BF_NP)
    for c in range(C):
        W3r[c, :, 16 * (c // 4), :] = W3.reshape(C, 2, P)[c].astype(BF_NP)
    b1h = np.ascontiguousarray(b1.reshape(C * 2, P).T)  # [128, C*2] f32
    b2h = np.ascontiguousarray(b2.reshape(C * 2, P).T)
    bias = np.concatenate([b1h, b2h], axis=1).astype(np.float32)  # [128, 32]
    boot = np.concatenate(
        [np.ascontiguousarray(W1p[0]),
         np.ascontiguousarray(bias).view(BF_NP)], axis=1)  # [128, 192] bf16
    return W1p, W2r, W3r, boot


def _make_in_maps(st, W1, b1, W2, b2, W3):
    W1p, W2r, W3r, boot = _prep_weights(W1, b1, W2, b2, W3)
    in_maps = []
    for core in range(NCORES):
        shard = st[core * NB:(core + 1) * NB]            # [8192, 64]
        stT = np.ascontiguousarray(
            np.concatenate([shard.T, shard.T], axis=0).astype(BF_NP))
        in_maps.append({"st": stT, "boot": boot, "w1": W1p, "w2": W2r,
                        "w3": W3r})
    return in_maps


class _SpmdExec:
    """Reusable jitted shard_map executor for a compiled Bass module
    (mirrors concourse.bass2jax.run_bass_via_pjrt; verified bit-identical)."""

    def __init__(self, nc, n_cores):
        import jax
        from jax.sharding import Mesh, PartitionSpec
        from jax.experimental.shard_map import shard_map
        from concourse.bass2jax import (_bass_exec_p, partition_id_tensor,
                                        install_neuronx_cc_hook)

        install_neuronx_cc_hook()
        self.n_cores = n_cores
        in_names, out_names, out_avals = [], [], []
        pname = nc.partition_id_tensor.name if nc.partition_id_tensor else None
        for alloc in nc.m.functions[0].allocations:
            if not isinstance(alloc, mybir.MemoryLocationSet):
                continue
            name = alloc.memorylocations[0].name
            if alloc.kind == "ExternalInput":
                if name != pname:
                    in_names.append(name)
            elif alloc.kind == "ExternalOutput":
                out_names.append(name)
                out_avals.append(jax.core.ShapedArray(
                    tuple(alloc.tensor_shape), mybir.dt.np(alloc.dtype)))
        self.in_names, self.out_names, self.out_avals = \
            in_names, out_names, out_avals
        all_in = in_names + out_names + ([pname] if pname else [])

        def _bdy(*args):
            ops = list(args)
            if pname is not None:
                ops.append(partition_id_tensor())
            return tuple(_bass_exec_p.bind(
                *ops, out_avals=tuple(out_avals), in_names=tuple(all_in),
                out_names=tuple(out_names), lowering_input_output_aliases=(),
                sim_require_finite=True, sim_require_nnan=True, nc=nc))

        mesh = Mesh(np.asarray(jax.devices()[:n_cores]), ("core",))
        nio = len(in_names) + len(out_names)
        self.sharded = jax.jit(
            shard_map(_bdy, mesh=mesh,
                      in_specs=(PartitionSpec("core"),) * nio,
                      out_specs=(PartitionSpec("core"),) * len(out_names),
                      check_rep=False),
            keep_unused=True)

    def run(self, in_maps):
        args = [np.concatenate([np.asarray(m[n]) for m in in_maps], axis=0)
                for n in self.in_names]
        args += [np.zeros((self.n_cores * a.shape[0], *a.shape[1:]), a.dtype)
                 for a in self.out_avals]
        outs = self.sharded(*args)
        return [{n: np.asarray(outs[i]).reshape(
                    self.n_cores, *self.out_avals[i].shape)[c]
                 for i, n in enumerate(self.out_names)}
                for c in range(self.n_cores)]


_EXEC_CACHE = {}


def _run_spmd(nc, in_maps, first_call):
    """First call goes through bass_utils.run_bass_kernel_spmd; later calls
    reuse a cached PJRT executable (bit-identical output, no re-jit)."""
    if not first_call:
        ex = _EXEC_CACHE.get(id(nc))
        if ex is None:
            ex = _EXEC_CACHE[id(nc)] = _SpmdExec(nc, NCORES)
        return ex.run(in_maps)
    import os
    try:
        return run_bass_kernel_spmd(
            nc, in_maps, core_ids=list(range(NCORES))).results
    except ModuleNotFoundError:
        # BASS_TRACE set but the axon NTFF hook module is absent: force
        # trace off and retry.
        os.environ["BASS_NEVER_TRACE"] = "1"
        return run_bass_kernel_spmd(
            nc, in_maps, core_ids=list(range(NCORES))).results


_CALLED = False


def kernel(st, W1, b1, W2, b2, W3, b3):
    global _CALLED
    st = np.ascontiguousarray(np.asarray(st, np.float32))
    in_maps = _make_in_maps(
        st,
        np.asarray(W1, np.float32), np.asarray(b1, np.float32),
        np.asarray(W2, np.float32), np.asarray(b2, np.float32),
        np.asarray(W3, np.float32))
    nc = _build_nc(1)
    results = _run_spmd(nc, in_maps, first_call=not _CALLED)
    _CALLED = True

    b3v = np.asarray(b3, np.float32).reshape(1, C)
    # d row r holds expert 4 * (r % 2) + r // 2 (PSUM partition 16 * r maps
    # to col group r // 2, quad r % 2).
    perm = [4 * (r % 2) + r // 2 for r in range(C)]
    out = np.empty((B, C, 1), np.float32)
    for core in range(NCORES):
        d = results[core]["d"]                            # [8, 8192]
        out[core * NB:(core + 1) * NB, :, 0][:, perm] = d.T
    out[:, :, 0] += b3v
    return out


# revision 38
# speedup vs baseline: 1.3454x; 1.1050x over previous
"""Trainium2 Bass kernel for nn_MoEDiscriminator (8 experts, MLP 64->256->256->1).

Strategy (data-parallel over 8 NeuronCores):
- st [65536, 64] is sharded along batch: 8192 rows per core; expert weights
  are replicated on every core.
- All matmul operands are bf16 (host-converted); PSUM accumulates fp32.
  Measured end-to-end error vs the fp32 reference is ~4e-3 relative.
- Activations live as [feature_on_partitions, batch_on_free] SBUF tiles.
- Per expert c, per batch tile of 512:
    L1: K=64, so the two hidden halves run as a row-tiled PAIR: half 0 at
        PE rows 0-63, half 1 at rows 64-127 (st is duplicated across the
        two row groups). The two matmuls execute concurrently -> 2x L1.
    L2: 4 full 128x128 matmuls (2 out-halves x 2 k-chunks).
    L3: M=1 per expert; experts are col-tiled 4-at-a-time at PE columns
        0/32/64/96 (round-robin issue) -> 4 concurrent matmuls -> 4x L3.
        The [experts-on-partitions 0/32/64/96, 512] PSUM tile is DMA'd
        straight to DRAM with a partition-strided access pattern.
- relu(x + b) evictions run on ScalarE/VectorE, load-balanced; b3 is added
  on the host; output is reassembled host-side to [65536, 8, 1].
"""

import sys

sys.path.insert(0, "/opt/trn_rl_repo")
from contextlib import ExitStack

import numpy as np
import ml_dtypes

import concourse.bass as bass
import concourse.tile as tile
from concourse import bacc, mybir
from concourse.bass import ts
from concourse.bass_utils import run_bass_kernel_spmd

P = 128
C = 8            # experts
DS = 64          # input feature dim
H = 256          # hidden width
B = 65536        # full batch
NCORES = 8
NB = B // NCORES  # 8192 rows per core
BT = 512         # batch tile (free dim of matmuls)
NT = NB // BT    # 16
ST_CHUNKS = (512, 1536, 2048, 4096)   # graduated st chunk widths
PSUM_BUFS = (4, 3, 1)                 # (psumA, psumB, psumD)
H1_BUFS = 4
H2_BUFS = 10
ACT_EXTRA = 0     # if > 0: every ACT_EXTRA-th item, ACT also takes L2 h1 evict

f32 = mybir.dt.float32
bf16 = mybir.dt.bfloat16
AF = mybir.ActivationFunctionType
ALU = mybir.AluOpType
BF_NP = ml_dtypes.bfloat16

_NC_CACHE = {}


def _build_nc(repeats=1):
    key = (repeats, ST_CHUNKS, PSUM_BUFS, H1_BUFS, H2_BUFS, ACT_EXTRA)
    if key in _NC_CACHE:
        return _NC_CACHE[key]
    nc = bacc.Bacc("TRN2", target_bir_lowering=False, debug=False,
                   num_devices=NCORES)
    st_d = nc.dram_tensor("st", [P, NB], bf16, kind="ExternalInput").ap()
    # boot: W1pack[:, 0, :] ++ bitcast(b1h ++ b2h)
    boot_d = nc.dram_tensor("boot", [P, P + 4 * C * 2],
                            bf16, kind="ExternalInput").ap()
    w1_d = nc.dram_tensor("w1", [C, P, P], bf16, kind="ExternalInput").ap()
    w2_d = nc.dram_tensor("w2", [C, 2, 2, P, P], bf16,
                          kind="ExternalInput").ap()
    # w3[c, k, m, p]: expert c's k-chunk in column 16 * (c // 4) of a 32-wide
    # zero-padded stationary (full-width so the start=True matmul initializes
    # every partition of the bank). Both expert-quads accumulate into one PSUM bank:
    # expert c lands on partition 32 * (c % 4) + 16 * (c // 4), so a single
    # stride-16 partition copy evicts all 8 experts.
    w3_d = nc.dram_tensor("w3", [C, 2, 32, P], bf16, kind="ExternalInput").ap()
    d_d = nc.dram_tensor("d", [C, NB], f32, kind="ExternalOutput").ap()

    with tile.TileContext(nc) as tc, ExitStack() as ctx:
        const = ctx.enter_context(tc.tile_pool(name="const", bufs=2))
        work1 = ctx.enter_context(tc.tile_pool(name="work1", bufs=H1_BUFS))
        work2 = ctx.enter_context(tc.tile_pool(name="work2", bufs=H2_BUFS))
        psumA = ctx.enter_context(
            tc.tile_pool(name="psumA", bufs=PSUM_BUFS[0], space="PSUM"))
        psumB = ctx.enter_context(
            tc.tile_pool(name="psumB", bufs=PSUM_BUFS[1], space="PSUM"))
        psumD = ctx.enter_context(
            tc.tile_pool(name="psumD", bufs=PSUM_BUFS[2], space="PSUM"))

        def body():
            # Boot DMA first: expert-0 W1 pair + both bias tables, so the
            # first L1 matmul waits on a single small transfer.
            boot_sb = const.tile([P, P + 4 * C * 2], bf16)
            nc.sync.dma_start(boot_sb[:], boot_d)
            bias_f32 = boot_sb[:, P:P + 4 * C * 2].bitcast(f32)  # [128, 32]
            b1_sb = bias_f32[:, 0:2 * C]
            b2_sb = bias_f32[:, 2 * C:4 * C]

            st_sb, st_off = [], []
            off = 0
            for i, cols in enumerate(ST_CHUNKS):
                st_sb.append(const.tile([P, cols], bf16, name=f"st_sb{i}"))
                st_off.append(off)
                off += cols
            assert off == NB
            nc.sync.dma_start(st_sb[0][:], st_d[:, 0:ST_CHUNKS[0]])

            w1_sb = const.tile([P, C, P], bf16)
            w2_sb = const.tile([P, C, 2, 2, P], bf16)
            w3_sb = const.tile([P, C, 2, 32], bf16)
            d_sb = const.tile([P, NB], f32)  # experts live on partitions 16*r
            # Weight DMAs interleaved so w1[c]/w2[c] land before expert c's
            # L1/L2 reach them in the pipeline.
            nc.sync.dma_start(w1_sb[:, 1:4], w1_d[1:4].rearrange("c p f -> p c f"))
            nc.sync.dma_start(w2_sb[:, 0:1],
                              w2_d[0:1].rearrange("c k j p f -> p c k j f"))
            nc.sync.dma_start(w1_sb[:, 4:8], w1_d[4:8].rearrange("c p f -> p c f"))
            nc.sync.dma_start(w3_sb[:], w3_d.rearrange("c k m p -> p c k m"))
            nc.sync.dma_start(w2_sb[:, 1:2],
                              w2_d[1:2].rearrange("c k j p f -> p c k j f"))
            nc.sync.dma_start(w2_sb[:, 2:4],
                              w2_d[2:4].rearrange("c k j p f -> p c k j f"))
            nc.sync.dma_start(st_sb[1][:],
                              st_d[:, st_off[1]:st_off[1] + ST_CHUNKS[1]])
            nc.sync.dma_start(w2_sb[:, 4:6],
                              w2_d[4:6].rearrange("c k j p f -> p c k j f"))
            nc.sync.dma_start(w2_sb[:, 6:8],
                              w2_d[6:8].rearrange("c k j p f -> p c k j f"))
            for i in range(2, len(ST_CHUNKS)):
                nc.sync.dma_start(st_sb[i][:],
                                  st_d[:, st_off[i]:st_off[i] + ST_CHUNKS[i]])

            def st_slice(t, r):
                col = t * BT
                for i, o in enumerate(st_off):
                    if o <= col < o + ST_CHUNKS[i]:
                        return st_sb[i][64 * r:64 * r + 64,
                                        col - o:col - o + BT]
                raise AssertionError

            def w1_ap(c, r):
                if c == 0:
                    return boot_sb[64 * r:64 * r + 64, 0:P]
                return w1_sb[64 * r:64 * r + 64, c, :]

            h1s, h2s = {}, {}

            def stage_l1(i):
                t, c = divmod(i, C)
                pA = [psumA.tile([P, BT], f32, tag="pA", name=f"pA{r}")
                      for r in range(2)]
                for r in range(2):
                    nc.tensor.matmul(pA[r][:], w1_ap(c, r), st_slice(t, r),
                                     start=True, stop=True)
                h1 = [work1.tile([P, BT], bf16, tag="h1", name=f"h1_{r}")
                      for r in range(2)]
                nc.scalar.activation(h1[0][:], pA[0][:], AF.Relu,
                                     bias=b1_sb[:, 2 * c:2 * c + 1])
                nc.vector.tensor_scalar(h1[1][:], pA[1][:],
                                        b1_sb[:, 2 * c + 1:2 * c + 2],
                                        0.0, ALU.add, ALU.max)
                h1s[i] = h1

            def stage_l2(i):
                t, c = divmod(i, C)
                h1 = h1s.pop(i)
                pB = [psumB.tile([P, BT], f32, tag="pB", name=f"pB{j}")
                      for j in range(2)]
                for j in range(2):
                    for k in range(2):
                        nc.tensor.matmul(pB[j][:], w2_sb[:, c, k, j, :],
                                         h1[k][:],
                                         start=(k == 0), stop=(k == 1))
                h2 = [work2.tile([P, BT], bf16, tag="h2", name=f"h2_{j}")
                      for j in range(2)]
                nc.scalar.activation(h2[0][:], pB[0][:], AF.Relu,
                                     bias=b2_sb[:, 2 * c:2 * c + 1])
                if ACT_EXTRA and i % ACT_EXTRA == 0:
                    nc.scalar.activation(h2[1][:], pB[1][:], AF.Relu,
                                         bias=b2_sb[:, 2 * c + 1:2 * c + 2])
                else:
                    nc.vector.tensor_scalar(h2[1][:], pB[1][:],
                                            b2_sb[:, 2 * c + 1:2 * c + 2],
                                            0.0, ALU.add, ALU.max)
                h2s[i] = h2

            pDs = {}

            def stage_l3_quad(m):
                # m = item index of the quad's last expert (c % 4 == 3)
                t, c_last = divmod(m, C)
                q = c_last // 4
                if q == 0:
                    pDs[t] = psumD.tile([P, BT], f32, tag="pD", name="pD")
                pD = pDs[t]
                for j in range(4):
                    for k in range(2):
                        e = 4 * q + j
                        h2 = h2s[t * C + e]
                        nc.tensor.matmul(pD[32 * j:32 * j + 32, :],
                                         w3_sb[:, e, k, :], h2[k][:],
                                         start=(q == 0 and k == 0),
                                         stop=(q == 1 and k == 1),
                                         tile_position=(0, 32 * j),
                                         skip_group_check=True)
                for j in range(4):
                    h2s.pop(t * C + 4 * q + j)
                if q == 1:
                    pD = pDs.pop(t)
                    if t % 2 == 0:
                        nc.scalar.copy(d_sb[0:113, ts(t, BT)], pD[0:113, :])
                    else:
                        nc.vector.tensor_copy(d_sb[0:113, ts(t, BT)],
                                              pD[0:113, :])

            N = NT * C
            for i in range(N):
                stage_l1(i)
                if i >= 1:
                    stage_l2(i - 1)
                m = i - 2
                if m >= 0 and m % 4 == 3:
                    stage_l3_quad(m)
            stage_l2(N - 1)
            stage_l3_quad(N - 1)
            for r in range(C):
                nc.sync.dma_start(d_d[r:r + 1, :], d_sb[16 * r:16 * r + 1, :])

        for _rep in range(repeats):
            body()

    nc.compile()
    _NC_CACHE[key] = nc
    return nc


def _prep_weights(W1, b1, W2, b2, W3):
    # W1p[c]: rows 0-63 = W1[c, :, 0:128], rows 64-127 = W1[c, :, 128:256]
    W1p = np.empty((C, P, P), BF_NP)
    for c in range(C):
        W1p[c, 0:DS, :] = W1[c][:, 0:P].astype(BF_NP)
        W1p[c, DS:P, :] = W1[c][:, P:2 * P].astype(BF_NP)
    W2r = np.ascontiguousarray(
        W2.reshape(C, 2, P, 2, P).transpose(0, 1, 3, 2, 4)
    ).astype(BF_NP)  # W2r[c,k,j] = W2[c, k*128:(k+1)*128, j*128:(j+1)*128]
    # W3r[c, k, m, :]: expert c's k-chunk in column 16 * (c // 4) of a
    # 32-wide zero-padded stationary (see w3_d comment in _build_nc).
    W3r = np.zeros((C, 2, 32, P), BF_NP)
    for c in range(C):
        W3r[c, :, 16 * (c // 4), :] = W3.reshape(C, 2, P)[c].astype(BF_NP)
    b1h = np.ascontiguousarray(b1.reshape(C * 2, P).T)  # [128, C*2] f32
    b2h = np.ascontiguousarray(b2.reshape(C * 2, P).T)
    bias = np.concatenate([b1h, b2h], axis=1).astype(np.float32)  # [128, 32]
    boot = np.concatenate(
        [np.ascontiguousarray(W1p[0]),
         np.ascontiguousarray(bias).view(BF_NP)], axis=1)  # [128, 192] bf16
    return W1p, W2r, W3r, boot


def _make_in_maps(st, W1, b1, W2, b2, W3):
    W1p, W2r, W3r, boot = _prep_weights(W1, b1, W2, b2, W3)
    in_maps = []
    for core in range(NCORES):
        shard = st[core * NB:(core + 1) * NB]            # [8192, 64]
        stT = np.ascontiguousarray(
            np.concatenate([shard.T, shard.T], axis=0).astype(BF_NP))
        in_maps.append({"st": stT, "boot": boot, "w1": W1p, "w2": W2r,
                        "w3": W3r})
    return in_maps


class _SpmdExec:
    """Reusable jitted shard_map executor for a compiled Bass module
    (mirrors concourse.bass2jax.run_bass_via_pjrt; verified bit-identical)."""

    def __init__(self, nc, n_cores):
        import jax
        from jax.sharding import Mesh, PartitionSpec
        from jax.experimental.shard_map import shard_map
        from concourse.bass2jax import (_bass_exec_p, partition_id_tensor,
                                        install_neuronx_cc_hook)

        install_neuronx_cc_hook()
        self.n_cores = n_cores
        in_names, out_names, out_avals = [], [], []
        pname = nc.partition_id_tensor.name if nc.partition_id_tensor else None
        for alloc in nc.m.functions[0].allocations:
            if not isinstance(alloc, mybir.MemoryLocationSet):
                continue
            name = alloc.memorylocations[0].name
            if alloc.kind == "ExternalInput":
                if name != pname:
                    in_names.append(name)
            elif alloc.kind == "ExternalOutput":
                out_names.append(name)
                out_avals.append(jax.core.ShapedArray(
                    tuple(alloc.tensor_shape), mybir.dt.np(alloc.dtype)))
        self.in_names, self.out_names, self.out_avals = \
            in_names, out_names, out_avals
        all_in = in_names + out_names + ([pname] if pname else [])

        def _bdy(*args):
            ops = list(args)
            if pname is not None:
                ops.append(partition_id_tensor())
            return tuple(_bass_exec_p.bind(
                *ops, out_avals=tuple(out_avals), in_names=tuple(all_in),
                out_names=tuple(out_names), lowering_input_output_aliases=(),
                sim_require_finite=True, sim_require_nnan=True, nc=nc))

        mesh = Mesh(np.asarray(jax.devices()[:n_cores]), ("core",))
        nio = len(in_names) + len(out_names)
        self.sharded = jax.jit(
            shard_map(_bdy, mesh=mesh,
                      in_specs=(PartitionSpec("core"),) * nio,
                      out_specs=(PartitionSpec("core"),) * len(out_names),
                      check_rep=False),
            keep_unused=True)

    def run(self, in_maps):
        args = [np.concatenate([np.asarray(m[n]) for m in in_maps], axis=0)
                for n in self.in_names]
        args += [np.zeros((self.n_cores * a.shape[0], *a.shape[1:]), a.dtype)
                 for a in self.out_avals]
        outs = self.sharded(*args)
        return [{n: np.asarray(outs[i]).reshape(
                    self.n_cores, *self.out_avals[i].shape)[c]
                 for i, n in enumerate(self.out_names)}
                for c in range(self.n_cores)]


_EXEC_CACHE = {}


def _run_spmd(nc, in_maps, first_call):
    """First call goes through bass_utils.run_bass_kernel_spmd; later calls
    reuse a cached PJRT executable (bit-identical output, no re-jit)."""
    if not first_call:
        ex = _EXEC_CACHE.get(id(nc))
        if ex is None:
            ex = _EXEC_CACHE[id(nc)] = _SpmdExec(nc, NCORES)
        return ex.run(in_maps)
    import os
    try:
        return run_bass_kernel_spmd(
            nc, in_maps, core_ids=list(range(NCORES))).results
    except ModuleNotFoundError:
        # BASS_TRACE set but the axon NTFF hook module is absent: force
        # trace off and retry.
        os.environ["BASS_NEVER_TRACE"] = "1"
        return run_bass_kernel_spmd(
            nc, in_maps, core_ids=list(range(NCORES))).results


_CALLED = False


def kernel(st, W1, b1, W2, b2, W3, b3):
    global _CALLED
    st = np.ascontiguousarray(np.asarray(st, np.float32))
    in_maps = _make_in_maps(
        st,
        np.asarray(W1, np.float32), np.asarray(b1, np.float32),
        np.asarray(W2, np.float32), np.asarray(b2, np.float32),
        np.asarray(W3, np.float32))
    nc = _build_nc(1)
    results = _run_spmd(nc, in_maps, first_call=not _CALLED)
    _CALLED = True

    b3v = np.asarray(b3, np.float32).reshape(1, C)
    # d row r holds expert 4 * (r % 2) + r // 2 (PSUM partition 16 * r maps
    # to col group r // 2, quad r % 2).
    perm = [4 * (r % 2) + r // 2 for r in range(C)]
    out = np.empty((B, C, 1), np.float32)
    for core in range(NCORES):
        d = results[core]["d"]                            # [8, 8192]
        out[core * NB:(core + 1) * NB, :, 0][:, perm] = d.T
    out[:, :, 0] += b3v
    return out
